# revision 4
# baseline (speedup 1.0000x reference)
"""DeepSeekMoE (top-2 of 8 experts + 2x shared expert) on 8 Trainium2 NeuronCores.

Strategy (hardcoded for x:[4,2048,2048], D=2048, H=1024, E=8, K=2):
  - Host: gating (logits/top-2/softmax) in float64 numpy; expert-parallel
    dispatch -- core e receives the tokens routed to expert e, gathered,
    transposed to [D, C] and padded to a common capacity C.  Shared-expert
    work is data-parallel: core c gets tokens [1024c, 1024(c+1)).
  - Device (SPMD): all matmuls in fp8 e4m3 with MatmulPerfMode.DoubleRow
    (K=256 contraction per instruction at 0.5 cycles/row -- ~4x fp16 MAC
    throughput).  Weights are pre-scaled by power-of-2 factors so their
    ~5e-4 magnitudes land in e4m3's normal range; the scale is divided
    out exactly in the ACT gelu (layer 1) or on the host (layer 2).
    Fully transposed dataflow (outputs [D, tokens]):
      hT = gelu(w1s.T @ xT + b1)  [fp8]; yrT = w2s.T @ hT   (routed)
      hsT = gelu(sw1s.T @ xsT + sb1);    ysT = sw2s.T @ hsT (shared)
    Layer-2 results are stored as float16 carrying the 2^s2 weight scale;
    the host divides it out in float64 during the combine.
  - Host: out[tok] += gate * (yr/s2 + b2[e]) scatter per expert; += ys/ss2 + sb2.

End-to-end absmax relative error vs float64 reference: ~1.2e-3 (fp8
operand rounding; gate is 2e-2).
"""
import contextlib
import os
import sys
import numpy as np

for _p in ("/root/.axon_site/_ro/trn_rl_repo", "/root/.axon_site/_ro/pypackages",
           "/opt/trn_rl_repo", "/opt/pypackages"):
    if os.path.isdir(_p) and _p not in sys.path:
        sys.path.append(_p)

import ml_dtypes
from concourse import bacc, mybir
from concourse import tile
from concourse.bass_utils import run_bass_kernel_spmd

NPF8 = ml_dtypes.float8_e4m3

# ---- problem constants (hardcoded per spec) ----
B, S, D, H, E = 4, 2048, 2048, 1024, 8
SH = 2 * H
N = B * S                    # 8192 tokens
TOPK = 2
NCORES = 8
TS = N // NCORES             # shared-expert tokens per core (1024)
ND = D // 128                # 16 contraction chunks over D
NH = H // 128                # 8 chunks over H
NSH = SH // 128              # 16 chunks over SH
TILE_N = 512                 # token-tile width (one PSUM bank of fp32)
NTS = TS // TILE_N           # shared-expert token tiles per core (2)

STRATEGY = "fp8"

F32 = mybir.dt.float32
F16 = mybir.dt.float16
FP8 = mybir.dt.float8e4
DRMODE = mybir.MatmulPerfMode.DoubleRowSwInterleave

LAST_RESULTS = None          # BassKernelResults of the most recent device run

_BUILD_CACHE = {}


def _swiv(wv, m):
    """SWI lhsT view of pair m of a [128, K, 128] weight tile: [128, 2, 128]
    with dim1=interleave index (stride 1), dim2=reversed column (stride 2)."""
    return (wv[:, 2 * m:2 * m + 2, :].rearrange("p a t -> p (a t)")
            .rearrange("p (t i) -> p i t", i=2))


def _ntiles(total):
    """(offset, width) token tiles covering `total` (512s, then remainder)."""
    out = []
    t = 0
    while t < total:
        w = min(TILE_N, total - t)
        out.append((t, w))
        t += w
    return out


def _build(C, strategy="fp8", loop_iters=None):
    """Build + compile the per-core SPMD program for capacity C."""
    no_store = os.environ.get("MOE_NO_STORE") == "1"     # bench-only knob
    dup_r = int(os.environ.get("MOE_DUP_R", "1"))        # bench-only knob
    dup_s = int(os.environ.get("MOE_DUP_S", "1"))        # bench-only knob
    key = (C, strategy, loop_iters, no_store, dup_r, dup_s)
    if key in _BUILD_CACHE:
        return _BUILD_CACHE[key]

    nc = bacc.Bacc("TRN2", target_bir_lowering=False, debug=False)

    # Weights: per-column-tile layout [ncols, 128(p), kchunks*128], element
    # (col, p, kc*128+c) = w[kc*128 + p, col*128 + c] (pre-scaled, fp8).
    # x: flat fp8, blocked per token tile: block(n) is [128(p), ND*nw] with
    # element (p, d*nw+c) = xT2d[d*128 + p, n0 + c].
    # y: flat f16, blocked per (token tile, dt): tile block [128(p), nw] =
    # yrT2d[dt*128 + p, n0:n0+nw] (carries the 2^s2 scale).
    d_xT = nc.dram_tensor("xT", [D * C], FP8, kind="ExternalInput")
    d_w1 = nc.dram_tensor("w1", [NH, 128, ND * 128], FP8, kind="ExternalInput")
    d_w2 = nc.dram_tensor("w2", [ND, 128, NH * 128], FP8, kind="ExternalInput")
    d_b1 = nc.dram_tensor("b1c", [H, 1], F32, kind="ExternalInput")
    d_xsT = nc.dram_tensor("xsT", [NTS, 128, ND * TILE_N], FP8, kind="ExternalInput")
    d_sw1 = nc.dram_tensor("sw1", [NSH, 128, ND * 128], FP8, kind="ExternalInput")
    d_sw2 = nc.dram_tensor("sw2", [ND, 128, NSH * 128], FP8, kind="ExternalInput")
    d_sb1 = nc.dram_tensor("sb1c", [SH, 1], F32, kind="ExternalInput")
    d_yrT = nc.dram_tensor("yrT", [D * C], F16, kind="ExternalOutput")
    d_ysT = nc.dram_tensor("ysT", [NTS, ND, 128, TILE_N], F16, kind="ExternalOutput")

    v_b1 = d_b1.ap().rearrange("(b p) o -> p b o", p=128)     # [128, NH, 1]
    v_sb1 = d_sb1.ap().rearrange("(b p) o -> p b o", p=128)   # [128, NSH, 1]

    def x_tile_view(n0, nw):
        a = d_xT.ap()[D * n0: D * (n0 + nw)]
        return a.rearrange("(p d c) -> p d c", p=128, d=ND)

    def xs_tile_view(n):
        return d_xsT.ap()[n].rearrange("p (d c) -> p d c", d=ND)

    def yr_tile_view(n0, nw, dt):
        a = d_yrT.ap()[D * n0 + dt * 128 * nw: D * n0 + (dt + 1) * 128 * nw]
        return a.rearrange("(p c) -> p c", p=128)

    def wcol_view(d_w, col, kchunks):
        return d_w.ap()[col].rearrange("p (k c) -> p k c", k=kchunks)

    gelu = mybir.ActivationFunctionType.Gelu
    rtiles = _ntiles(C)

    with tile.TileContext(nc) as tc:
        with tc.tile_pool(name="wres", bufs=1) as wres, \
             tc.tile_pool(name="xs", bufs=3) as xs, \
             tc.tile_pool(name="hp", bufs=2) as hp, \
             tc.tile_pool(name="hs", bufs=2) as hsp, \
             tc.tile_pool(name="bias", bufs=1) as bias, \
             tc.tile_pool(name="ps1", bufs=4, space="PSUM") as ps1, \
             tc.tile_pool(name="ps2", bufs=4, space="PSUM") as ps2, \
             tc.tile_pool(name="st", bufs=4) as stp:
          with (tc.For_i(0, loop_iters, 1) if loop_iters is not None
                else contextlib.nullcontext()):

            # biases first (tiny; the first gelu needs b1), then x tile 0.
            b1t = bias.tile([128, NH], F32, tag="b1")
            sb1t = bias.tile([128, NSH], F32, tag="sb1")
            nc.sync.dma_start(out=b1t[:], in_=v_b1[:, :, 0])
            nc.sync.dma_start(out=sb1t[:], in_=v_sb1[:, :, 0])

            # warm the ACT gelu table while DMAs stream: the auto-inserted
            # LoadActFuncSet binds to the first Activation in program order.
            warm = bias.tile([128, 1], F32, tag="warm")
            nc.vector.memset(warm[:], 0.0)
            nc.scalar.activation(warm[:], warm[:], gelu)

            xt0 = xs.tile([128, ND, TILE_N], FP8, tag="xstream")
            nc.sync.dma_start(out=xt0[:, :, :rtiles[0][1]],
                              in_=x_tile_view(*rtiles[0]))

            # routed weights resident (gpsimd queue; fine-grained deps so the
            # first matmul only waits on its own 0.25 MB column).
            w1cols = []
            for ht in range(NH):
                t = wres.tile([128, ND, 128], FP8, tag=f"w1c{ht}")
                nc.gpsimd.dma_start(out=t[:], in_=wcol_view(d_w1, ht, ND))
                w1cols.append(t)
            w2cols = []
            for dt in range(ND):
                t = wres.tile([128, NH, 128], FP8, tag=f"w2c{dt}")
                nc.gpsimd.dma_start(out=t[:], in_=wcol_view(d_w2, dt, NH))
                w2cols.append(t)
            # shared weights resident too, streamed during the routed phase
            # on the gpsimd queue behind the routed weights.
            sw1cols = []
            for st in range(NSH):
                t = wres.tile([128, ND, 128], FP8, tag=f"sw1c{st}")
                nc.gpsimd.dma_start(out=t[:], in_=wcol_view(d_sw1, st, ND))
                sw1cols.append(t)
            sw2cols = []
            for dt in range(ND):
                t = wres.tile([128, NSH, 128], FP8, tag=f"sw2c{dt}")
                nc.gpsimd.dma_start(out=t[:], in_=wcol_view(d_sw2, dt, NSH))
                sw2cols.append(t)

            _store_ctr = [0]

            def store(dram_ap, psum_ap, nw):
                if no_store:
                    return
                ot = stp.tile([128, TILE_N], F16, tag="stage")
                if _store_ctr[0] % 2 == 0:
                    nc.vector.tensor_copy(ot[:, :nw], psum_ap)
                    nc.sync.dma_start(out=dram_ap, in_=ot[:, :nw])
                else:
                    nc.scalar.activation(ot[:, :nw], psum_ap,
                                         mybir.ActivationFunctionType.Copy)
                    nc.scalar.dma_start(out=dram_ap, in_=ot[:, :nw])
                _store_ctr[0] += 1

            # ---- routed phase ----
            for n, (n0, nw) in enumerate(rtiles * dup_r):
                if n == 0:
                    xt = xt0
                else:
                    xt = xs.tile([128, ND, TILE_N], FP8, tag="xstream")
                    nc.sync.dma_start(out=xt[:, :, :nw], in_=x_tile_view(n0, nw))
                ht_t = hp.tile([128, NH, TILE_N], FP8, tag="h")
                for ht in range(NH):
                    wv = w1cols[ht]
                    pt = ps1.tile([128, TILE_N], F32, tag="p1")
                    for m in range(ND // 2):
                        nc.tensor.matmul(pt[:, :nw], lhsT=_swiv(wv, m),
                                         rhs=xt[:, 2 * m:2 * m + 2, :nw],
                                         perf_mode=DRMODE,
                                         start=(m == 0), stop=(m == ND // 2 - 1))
                    nc.scalar.activation(ht_t[:, ht, :nw], pt[:, :nw], gelu,
                                         bias=b1t[:, ht:ht + 1], scale=_S1INV[0])
                for dt in range(ND):
                    wv = w2cols[dt]
                    pt2 = ps2.tile([128, TILE_N], F32, tag="p2")
                    for m in range(NH // 2):
                        nc.tensor.matmul(pt2[:, :nw], lhsT=_swiv(wv, m),
                                         rhs=ht_t[:, 2 * m:2 * m + 2, :nw],
                                         perf_mode=DRMODE,
                                         start=(m == 0), stop=(m == NH // 2 - 1))
                    store(yr_tile_view(n0, nw, dt), pt2[:, :nw], nw)

            # ---- shared expert phase ----
            for _sdup in range(dup_s):
                for n in range(NTS):
                    xst = xs.tile([128, ND, TILE_N], FP8, tag="xstream")
                    nc.sync.dma_start(out=xst[:], in_=xs_tile_view(n))
                    hst = hsp.tile([128, NSH, TILE_N], FP8, tag="hs")
                    for st in range(NSH):
                        swv = sw1cols[st]
                        pt = ps1.tile([128, TILE_N], F32, tag="p1")
                        for m in range(ND // 2):
                            nc.tensor.matmul(pt[:], lhsT=_swiv(swv, m),
                                             rhs=xst[:, 2 * m:2 * m + 2, :],
                                             perf_mode=DRMODE,
                                             start=(m == 0), stop=(m == ND // 2 - 1))
                        nc.scalar.activation(hst[:, st, :], pt[:], gelu,
                                             bias=sb1t[:, st:st + 1], scale=_SS1INV[0])
                    for dt in range(ND):
                        swv2 = sw2cols[dt]
                        pt2 = ps2.tile([128, TILE_N], F32, tag="p2")
                        for m in range(NSH // 2):
                            nc.tensor.matmul(pt2[:], lhsT=_swiv(swv2, m),
                                             rhs=hst[:, 2 * m:2 * m + 2, :],
                                             perf_mode=DRMODE,
                                             start=(m == 0), stop=(m == NSH // 2 - 1))
                        store(d_ysT[n, dt, :, :], pt2[:], TILE_N)

    nc.compile()
    _BUILD_CACHE[key] = nc
    return nc


def _route(xf, gate_w):
    """float64 gating: top-2 indices (lax.top_k tie-break) + softmax gates."""
    logits = xf.astype(np.float64) @ np.asarray(gate_w).astype(np.float64)
    order = np.argsort(-logits, axis=1, kind="stable")
    idx = order[:, :TOPK]                                           # [N, 2]
    tl = np.take_along_axis(logits, idx, axis=1)
    tl = tl - tl.max(axis=1, keepdims=True)
    eg = np.exp(tl)
    gates = eg / eg.sum(axis=1, keepdims=True)                      # [N, 2]
    return idx, gates


def _p2scale(w):
    """Largest power of two s with absmax(w*s) <= 120 (e4m3 normal range)."""
    return 2.0 ** np.floor(np.log2(120.0 / np.abs(w).max()))


def _blockT(w, scale):
    """[K, M] weight -> scaled fp8 per-column-tile layout [M/128, 128(p), K]
    with each contraction pair m stored in DoubleRowSwInterleave order:
    free position (m, 2t+i) holds w[(2m+i)*128 + p, col*128 + (127-t)]*scale."""
    K, M = w.shape
    r = (np.asarray(w) * scale).astype(NPF8).reshape(K // 128, 128, M // 128, 128)
    b = np.ascontiguousarray(r.transpose(2, 1, 0, 3))      # [col, p, kchunk, c]
    b5 = b.reshape(M // 128, 128, K // 256, 2, 128)        # [col, p, m, i, c]
    out = np.empty((M // 128, 128, K // 256, 128, 2), NPF8)
    out[..., 0] = b5[:, :, :, 0, ::-1]
    out[..., 1] = b5[:, :, :, 1, ::-1]
    return np.ascontiguousarray(out).reshape(M // 128, 128, K)


def _pack_x(xT2d, tiles):
    """[D, C] fp8 -> flat blocked per tile: block(n) [128, ND*nw],
    (p, d*nw+c) = xT2d[d*128+p, n0+c]."""
    r = xT2d.reshape(ND, 128, xT2d.shape[1])
    parts = [np.ascontiguousarray(r[:, :, n0:n0 + nw].transpose(1, 0, 2)).ravel()
             for n0, nw in tiles]
    return np.concatenate(parts)


def _unpack_yr(flat, C):
    """Inverse of the yrT blocked layout -> [D, C] float32."""
    y = np.empty((D, C), np.float32)
    for n0, nw in _ntiles(C):
        y[:, n0:n0 + nw] = flat[D * n0: D * (n0 + nw)].reshape(D, nw)
    return y


# gelu input scales (1/s1, 1/ss1) are baked into the compiled program; they
# are set by _prepare before _build runs (module-level for _build to read).
_S1INV = [1.0]
_SS1INV = [1.0]
SCALES = {}


def _prepare(x, gate_w, w1, b1, w2, shared_w1, shared_b1, shared_w2, npdt=None):
    """Host routing + per-core input maps. Returns (C, in_maps, perm, gsel)."""
    xf = np.ascontiguousarray(np.asarray(x).reshape(N, D))
    idx, gates = _route(xf, gate_w)

    perm = []      # token ids routed to each expert (ascending)
    gsel = []      # matching gate weight
    for e in range(E):
        hit0 = idx[:, 0] == e
        hit1 = idx[:, 1] == e
        sel = np.where(hit0 | hit1)[0]
        g = np.where(hit0[sel], gates[sel, 0], gates[sel, 1])
        perm.append(sel)
        gsel.append(g)
    cmax = max(len(p) for p in perm)
    C = ((cmax + 127) // 128) * 128
    rtiles = _ntiles(C)
    stiles = _ntiles(TS)

    w1 = np.asarray(w1); w2 = np.asarray(w2)
    shared_w1 = np.asarray(shared_w1); shared_w2 = np.asarray(shared_w2)
    s1 = _p2scale(w1); s2 = _p2scale(w2)
    ss1 = _p2scale(shared_w1); ss2 = _p2scale(shared_w2)
    SCALES.update(s1=s1, s2=s2, ss1=ss1, ss2=ss2)
    _S1INV[0] = 1.0 / s1
    _SS1INV[0] = 1.0 / ss1

    xfc = xf.astype(NPF8)
    sw1b = _blockT(shared_w1, ss1)
    sw2b = _blockT(shared_w2, ss2)
    sb1c = np.ascontiguousarray(np.asarray(shared_b1).astype(np.float32)).reshape(SH, 1)
    in_maps = []
    for c in range(E):
        sel = perm[c]
        xT = np.zeros((D, C), NPF8)
        xT[:, :len(sel)] = xfc[sel].T
        xsT = np.ascontiguousarray(xfc[c * TS:(c + 1) * TS].T)
        in_maps.append({
            "xT": _pack_x(xT, rtiles),
            "w1": _blockT(w1[c], s1),
            "w2": _blockT(w2[c], s2),
            "b1c": np.ascontiguousarray(np.asarray(b1[c]).astype(np.float32)).reshape(H, 1),
            "xsT": _pack_x(xsT, stiles).reshape(NTS, 128, ND * TILE_N),
            "sw1": sw1b,
            "sw2": sw2b,
            "sb1c": sb1c,
        })
    return C, in_maps, perm, gsel


def kernel(x, gate_w, w1, b1, w2, b2, shared_w1, shared_b1, shared_w2, shared_b2):
    global LAST_RESULTS
    C, in_maps, perm, gsel = _prepare(
        x, gate_w, w1, b1, w2, shared_w1, shared_b1, shared_w2)
    nc = _build(C, STRATEGY)

    LAST_RESULTS = run_bass_kernel_spmd(nc, in_maps, core_ids=list(range(NCORES)))
    res = LAST_RESULTS.results

    b2 = np.asarray(b2)
    shared_b2 = np.asarray(shared_b2)
    s2inv = 1.0 / SCALES["s2"]
    ss2inv = 1.0 / SCALES["ss2"]
    out = np.zeros((N, D), np.float64)
    for c in range(E):
        sel = perm[c]
        yr = _unpack_yr(res[c]["yrT"], C).T[:len(sel)].astype(np.float64) * s2inv
        out[sel] += gsel[c][:, None] * (yr + b2[c].astype(np.float64))
        ys = res[c]["ysT"].astype(np.float32).reshape(NTS, D, TILE_N)
        ys2d = np.concatenate([ys[n] for n in range(NTS)], axis=1)  # [D, TS]
        out[c * TS:(c + 1) * TS] += (ys2d.T.astype(np.float64) * ss2inv
                                     + shared_b2.astype(np.float64))

    return out.reshape(B, S, D).astype(np.float32)


# revision 5
# speedup vs baseline: 2.1186x; 2.1186x over previous
"""DeepSeekMoE (top-2 of 8 experts + 2x shared expert) on 8 Trainium2 NeuronCores.

Strategy (hardcoded for x:[4,2048,2048], D=2048, H=1024, E=8, K=2):
  - Host: gating (logits/top-2/softmax) in float64 numpy; expert-parallel
    dispatch -- core e receives the tokens routed to expert e, gathered,
    transposed to [D, C] and padded to a common capacity C.  Shared-expert
    work is data-parallel: core c gets tokens [1024c, 1024(c+1)).
  - Device (SPMD): all matmuls in fp8 e4m3 with MatmulPerfMode.DoubleRow
    (K=256 contraction per instruction at 0.5 cycles/row -- ~4x fp16 MAC
    throughput).  Weights are pre-scaled by power-of-2 factors so their
    ~5e-4 magnitudes land in e4m3's normal range; the scale is divided
    out exactly in the ACT gelu (layer 1) or on the host (layer 2).
    Fully transposed dataflow (outputs [D, tokens]):
      hT = gelu(w1s.T @ xT + b1)  [fp8]; yrT = w2s.T @ hT   (routed)
      hsT = gelu(sw1s.T @ xsT + sb1);    ysT = sw2s.T @ hsT (shared)
    Layer-2 results are stored as float16 carrying the 2^s2 weight scale;
    the host divides it out in float64 during the combine.
  - Host: out[tok] += gate * (yr/s2 + b2[e]) scatter per expert; += ys/ss2 + sb2.

End-to-end absmax relative error vs float64 reference: ~1.2e-3 (fp8
operand rounding; gate is 2e-2).
"""
import contextlib
import os
import sys
import numpy as np

for _p in ("/root/.axon_site/_ro/trn_rl_repo", "/root/.axon_site/_ro/pypackages",
           "/opt/trn_rl_repo", "/opt/pypackages"):
    if os.path.isdir(_p) and _p not in sys.path:
        sys.path.append(_p)

import ml_dtypes
from concourse import bacc, mybir
from concourse import tile
from concourse.bass_utils import run_bass_kernel_spmd

NPF8 = ml_dtypes.float8_e4m3

# ---- problem constants (hardcoded per spec) ----
B, S, D, H, E = 4, 2048, 2048, 1024, 8
SH = 2 * H
N = B * S                    # 8192 tokens
TOPK = 2
NCORES = 8
TS = N // NCORES             # shared-expert tokens per core (1024)
ND = D // 128                # 16 contraction chunks over D
NH = H // 128                # 8 chunks over H
NSH = SH // 128              # 16 chunks over SH
TILE_N = 512                 # token-tile width (one PSUM bank of fp32)
NTS = TS // TILE_N           # shared-expert token tiles per core (2)

STRATEGY = "fp8"

F32 = mybir.dt.float32
F16 = mybir.dt.float16
FP8 = mybir.dt.float8e4
DRMODE = mybir.MatmulPerfMode.DoubleRowSwInterleave

LAST_RESULTS = None          # BassKernelResults of the most recent device run

_BUILD_CACHE = {}


def _swiv(wv, m):
    """SWI lhsT view of pair m of a [128, K, 128] weight tile: [128, 2, 128]
    with dim1=interleave index (stride 1), dim2=reversed column (stride 2)."""
    return (wv[:, 2 * m:2 * m + 2, :].rearrange("p a t -> p (a t)")
            .rearrange("p (t i) -> p i t", i=2))


def _ntiles(total):
    """(offset, width) token tiles covering `total` (512s, then remainder)."""
    out = []
    t = 0
    while t < total:
        w = min(TILE_N, total - t)
        out.append((t, w))
        t += w
    return out


def _build(C, strategy="fp8", loop_iters=None):
    """Build + compile the per-core SPMD program for capacity C."""
    no_store = os.environ.get("MOE_NO_STORE") == "1"     # bench-only knob
    dup_r = int(os.environ.get("MOE_DUP_R", "1"))        # bench-only knob
    dup_s = int(os.environ.get("MOE_DUP_S", "1"))        # bench-only knob
    st8 = os.environ.get("MOE_ST8") == "1"               # fp8 output stores
    STDT = FP8 if st8 else F16
    key = (C, strategy, loop_iters, no_store, dup_r, dup_s, st8)
    if key in _BUILD_CACHE:
        return _BUILD_CACHE[key]

    nc = bacc.Bacc("TRN2", target_bir_lowering=False, debug=False)

    # Weights: per-column-tile layout [ncols, 128(p), kchunks*128], element
    # (col, p, kc*128+c) = w[kc*128 + p, col*128 + c] (pre-scaled, fp8).
    # x: flat fp8, blocked per token tile: block(n) is [128(p), ND*nw] with
    # element (p, d*nw+c) = xT2d[d*128 + p, n0 + c].
    # y: flat f16, blocked per (token tile, dt): tile block [128(p), nw] =
    # yrT2d[dt*128 + p, n0:n0+nw] (carries the 2^s2 scale).
    d_xT = nc.dram_tensor("xT", [D * C], FP8, kind="ExternalInput")
    d_w1 = nc.dram_tensor("w1", [NH, 128, ND * 128], FP8, kind="ExternalInput")
    d_w2 = nc.dram_tensor("w2", [ND, 128, NH * 128], FP8, kind="ExternalInput")
    d_b1 = nc.dram_tensor("b1c", [H, 1], F32, kind="ExternalInput")
    d_xsT = nc.dram_tensor("xsT", [NTS, 128, ND * TILE_N], FP8, kind="ExternalInput")
    d_sw1 = nc.dram_tensor("sw1", [NSH, 128, ND * 128], FP8, kind="ExternalInput")
    d_sw2 = nc.dram_tensor("sw2", [ND, 128, NSH * 128], FP8, kind="ExternalInput")
    d_sb1 = nc.dram_tensor("sb1c", [SH, 1], F32, kind="ExternalInput")
    d_yrT = nc.dram_tensor("yrT", [D * C], STDT, kind="ExternalOutput")
    d_ysT = nc.dram_tensor("ysT", [NTS, ND, 128, TILE_N], STDT, kind="ExternalOutput")

    v_b1 = d_b1.ap().rearrange("(b p) o -> p b o", p=128)     # [128, NH, 1]
    v_sb1 = d_sb1.ap().rearrange("(b p) o -> p b o", p=128)   # [128, NSH, 1]

    def x_tile_view(n0, nw):
        a = d_xT.ap()[D * n0: D * (n0 + nw)]
        return a.rearrange("(p d c) -> p d c", p=128, d=ND)

    def xs_tile_view(n):
        return d_xsT.ap()[n].rearrange("p (d c) -> p d c", d=ND)

    def yr_tile_view(n0, nw, dt):
        a = d_yrT.ap()[D * n0 + dt * 128 * nw: D * n0 + (dt + 1) * 128 * nw]
        return a.rearrange("(p c) -> p c", p=128)

    def wcol_view(d_w, col, kchunks):
        return d_w.ap()[col].rearrange("p (k c) -> p k c", k=kchunks)

    gelu = mybir.ActivationFunctionType.Gelu
    rtiles = _ntiles(C)

    with tile.TileContext(nc) as tc:
        with tc.tile_pool(name="wres", bufs=1) as wres, \
             tc.tile_pool(name="xs", bufs=3) as xs, \
             tc.tile_pool(name="hp", bufs=2) as hp, \
             tc.tile_pool(name="hs", bufs=2) as hsp, \
             tc.tile_pool(name="bias", bufs=1) as bias, \
             tc.tile_pool(name="ps1", bufs=4, space="PSUM") as ps1, \
             tc.tile_pool(name="ps2", bufs=4, space="PSUM") as ps2, \
             tc.tile_pool(name="st", bufs=4) as stp:
          with (tc.For_i(0, loop_iters, 1) if loop_iters is not None
                else contextlib.nullcontext()):

            # biases first (tiny; the first gelu needs b1), then x tile 0.
            b1t = bias.tile([128, NH], F32, tag="b1")
            sb1t = bias.tile([128, NSH], F32, tag="sb1")
            nc.sync.dma_start(out=b1t[:], in_=v_b1[:, :, 0])
            nc.sync.dma_start(out=sb1t[:], in_=v_sb1[:, :, 0])

            # warm the ACT gelu table while DMAs stream: the auto-inserted
            # LoadActFuncSet binds to the first Activation in program order.
            warm = bias.tile([128, 1], F32, tag="warm")
            nc.vector.memset(warm[:], 0.0)
            nc.scalar.activation(warm[:], warm[:], gelu)

            xt0 = xs.tile([128, ND, TILE_N], FP8, tag="xstream")
            nc.sync.dma_start(out=xt0[:, :, :rtiles[0][1]],
                              in_=x_tile_view(*rtiles[0]))

            # routed weights resident (gpsimd queue; fine-grained deps so the
            # first matmul only waits on its own 0.25 MB column).
            w1cols = []
            for ht in range(NH):
                t = wres.tile([128, ND, 128], FP8, tag=f"w1c{ht}")
                nc.gpsimd.dma_start(out=t[:], in_=wcol_view(d_w1, ht, ND))
                w1cols.append(t)
            w2cols = []
            for dt in range(ND):
                t = wres.tile([128, NH, 128], FP8, tag=f"w2c{dt}")
                nc.gpsimd.dma_start(out=t[:], in_=wcol_view(d_w2, dt, NH))
                w2cols.append(t)
            # shared weights resident too, streamed during the routed phase
            # on the gpsimd queue behind the routed weights.
            sw1cols = []
            for st in range(NSH):
                t = wres.tile([128, ND, 128], FP8, tag=f"sw1c{st}")
                nc.gpsimd.dma_start(out=t[:], in_=wcol_view(d_sw1, st, ND))
                sw1cols.append(t)
            sw2cols = []
            for dt in range(ND):
                t = wres.tile([128, NSH, 128], FP8, tag=f"sw2c{dt}")
                nc.gpsimd.dma_start(out=t[:], in_=wcol_view(d_sw2, dt, NSH))
                sw2cols.append(t)

            _store_ctr = [0]

            def store(dram_ap, psum_ap, nw):
                if no_store:
                    return
                ot = stp.tile([128, TILE_N], STDT, tag="stage")
                if _store_ctr[0] % 2 == 0:
                    nc.vector.tensor_copy(ot[:, :nw], psum_ap)
                    nc.sync.dma_start(out=dram_ap, in_=ot[:, :nw])
                else:
                    nc.scalar.activation(ot[:, :nw], psum_ap,
                                         mybir.ActivationFunctionType.Copy)
                    nc.scalar.dma_start(out=dram_ap, in_=ot[:, :nw])
                _store_ctr[0] += 1

            # ---- routed phase ----
            for n, (n0, nw) in enumerate(rtiles * dup_r):
                if n == 0:
                    xt = xt0
                else:
                    xt = xs.tile([128, ND, TILE_N], FP8, tag="xstream")
                    nc.sync.dma_start(out=xt[:, :, :nw], in_=x_tile_view(n0, nw))
                ht_t = hp.tile([128, NH, TILE_N], FP8, tag="h")
                for ht in range(NH):
                    wv = w1cols[ht]
                    pt = ps1.tile([128, TILE_N], F32, tag="p1")
                    for m in range(ND // 2):
                        nc.tensor.matmul(pt[:, :nw], lhsT=_swiv(wv, m),
                                         rhs=xt[:, 2 * m:2 * m + 2, :nw],
                                         perf_mode=DRMODE,
                                         start=(m == 0), stop=(m == ND // 2 - 1))
                    nc.scalar.activation(ht_t[:, ht, :nw], pt[:, :nw], gelu,
                                         bias=b1t[:, ht:ht + 1], scale=_S1INV[0])
                for dt in range(ND):
                    wv = w2cols[dt]
                    pt2 = ps2.tile([128, TILE_N], F32, tag="p2")
                    for m in range(NH // 2):
                        nc.tensor.matmul(pt2[:, :nw], lhsT=_swiv(wv, m),
                                         rhs=ht_t[:, 2 * m:2 * m + 2, :nw],
                                         perf_mode=DRMODE,
                                         start=(m == 0), stop=(m == NH // 2 - 1))
                    store(yr_tile_view(n0, nw, dt), pt2[:, :nw], nw)

            # ---- shared expert phase ----
            for _sdup in range(dup_s):
                for n in range(NTS):
                    xst = xs.tile([128, ND, TILE_N], FP8, tag="xstream")
                    nc.sync.dma_start(out=xst[:], in_=xs_tile_view(n))
                    hst = hsp.tile([128, NSH, TILE_N], FP8, tag="hs")
                    for st in range(NSH):
                        swv = sw1cols[st]
                        pt = ps1.tile([128, TILE_N], F32, tag="p1")
                        for m in range(ND // 2):
                            nc.tensor.matmul(pt[:], lhsT=_swiv(swv, m),
                                             rhs=xst[:, 2 * m:2 * m + 2, :],
                                             perf_mode=DRMODE,
                                             start=(m == 0), stop=(m == ND // 2 - 1))
                        nc.scalar.activation(hst[:, st, :], pt[:], gelu,
                                             bias=sb1t[:, st:st + 1], scale=_SS1INV[0])
                    for dt in range(ND):
                        swv2 = sw2cols[dt]
                        pt2 = ps2.tile([128, TILE_N], F32, tag="p2")
                        for m in range(NSH // 2):
                            nc.tensor.matmul(pt2[:], lhsT=_swiv(swv2, m),
                                             rhs=hst[:, 2 * m:2 * m + 2, :],
                                             perf_mode=DRMODE,
                                             start=(m == 0), stop=(m == NSH // 2 - 1))
                        store(d_ysT[n, dt, :, :], pt2[:], TILE_N)

    nc.compile()
    _BUILD_CACHE[key] = nc
    return nc


def _route(xf, gate_w):
    """float64 gating: top-2 indices (lax.top_k tie-break) + softmax gates."""
    logits = xf.astype(np.float64) @ np.asarray(gate_w).astype(np.float64)
    order = np.argsort(-logits, axis=1, kind="stable")
    idx = order[:, :TOPK]                                           # [N, 2]
    tl = np.take_along_axis(logits, idx, axis=1)
    tl = tl - tl.max(axis=1, keepdims=True)
    eg = np.exp(tl)
    gates = eg / eg.sum(axis=1, keepdims=True)                      # [N, 2]
    return idx, gates


def _p2scale(w):
    """Largest power of two s with absmax(w*s) <= 120 (e4m3 normal range)."""
    return 2.0 ** np.floor(np.log2(120.0 / np.abs(w).max()))


def _blockT(w, scale):
    """[K, M] weight -> scaled fp8 per-column-tile layout [M/128, 128(p), K]
    with each contraction pair m stored in DoubleRowSwInterleave order:
    free position (m, 2t+i) holds w[(2m+i)*128 + p, col*128 + (127-t)]*scale."""
    K, M = w.shape
    r = (np.asarray(w) * scale).astype(NPF8).reshape(K // 128, 128, M // 128, 128)
    b = np.ascontiguousarray(r.transpose(2, 1, 0, 3))      # [col, p, kchunk, c]
    b5 = b.reshape(M // 128, 128, K // 256, 2, 128)        # [col, p, m, i, c]
    out = np.empty((M // 128, 128, K // 256, 128, 2), NPF8)
    out[..., 0] = b5[:, :, :, 0, ::-1]
    out[..., 1] = b5[:, :, :, 1, ::-1]
    return np.ascontiguousarray(out).reshape(M // 128, 128, K)


def _pack_x(xT2d, tiles):
    """[D, C] fp8 -> flat blocked per tile: block(n) [128, ND*nw],
    (p, d*nw+c) = xT2d[d*128+p, n0+c]."""
    r = xT2d.reshape(ND, 128, xT2d.shape[1])
    parts = [np.ascontiguousarray(r[:, :, n0:n0 + nw].transpose(1, 0, 2)).ravel()
             for n0, nw in tiles]
    return np.concatenate(parts)


def _unpack_yr(flat, C):
    """Inverse of the yrT blocked layout -> [D, C] float32."""
    y = np.empty((D, C), np.float32)
    for n0, nw in _ntiles(C):
        y[:, n0:n0 + nw] = flat[D * n0: D * (n0 + nw)].reshape(D, nw)
    return y


# gelu input scales (1/s1, 1/ss1) are baked into the compiled program; they
# are set by _prepare before _build runs (module-level for _build to read).
_S1INV = [1.0]
_SS1INV = [1.0]
SCALES = {}


def _prepare(x, gate_w, w1, b1, w2, shared_w1, shared_b1, shared_w2, npdt=None):
    """Host routing + per-core input maps. Returns (C, in_maps, perm, gsel)."""
    xf = np.ascontiguousarray(np.asarray(x).reshape(N, D))
    idx, gates = _route(xf, gate_w)

    perm = []      # token ids routed to each expert (ascending)
    gsel = []      # matching gate weight
    for e in range(E):
        hit0 = idx[:, 0] == e
        hit1 = idx[:, 1] == e
        sel = np.where(hit0 | hit1)[0]
        g = np.where(hit0[sel], gates[sel, 0], gates[sel, 1])
        perm.append(sel)
        gsel.append(g)
    cmax = max(len(p) for p in perm)
    C = ((cmax + 127) // 128) * 128
    rtiles = _ntiles(C)
    stiles = _ntiles(TS)

    w1 = np.asarray(w1); w2 = np.asarray(w2)
    shared_w1 = np.asarray(shared_w1); shared_w2 = np.asarray(shared_w2)
    s1 = _p2scale(w1); s2 = _p2scale(w2)
    ss1 = _p2scale(shared_w1); ss2 = _p2scale(shared_w2)
    SCALES.update(s1=s1, s2=s2, ss1=ss1, ss2=ss2)
    _S1INV[0] = 1.0 / s1
    _SS1INV[0] = 1.0 / ss1

    xfc = xf.astype(NPF8)
    sw1b = _blockT(shared_w1, ss1)
    sw2b = _blockT(shared_w2, ss2)
    sb1c = np.ascontiguousarray(np.asarray(shared_b1).astype(np.float32)).reshape(SH, 1)
    in_maps = []
    for c in range(E):
        sel = perm[c]
        xT = np.zeros((D, C), NPF8)
        xT[:, :len(sel)] = xfc[sel].T
        xsT = np.ascontiguousarray(xfc[c * TS:(c + 1) * TS].T)
        in_maps.append({
            "xT": _pack_x(xT, rtiles),
            "w1": _blockT(w1[c], s1),
            "w2": _blockT(w2[c], s2),
            "b1c": np.ascontiguousarray(np.asarray(b1[c]).astype(np.float32)).reshape(H, 1),
            "xsT": _pack_x(xsT, stiles).reshape(NTS, 128, ND * TILE_N),
            "sw1": sw1b,
            "sw2": sw2b,
            "sb1c": sb1c,
        })
    return C, in_maps, perm, gsel


def kernel(x, gate_w, w1, b1, w2, b2, shared_w1, shared_b1, shared_w2, shared_b2):
    global LAST_RESULTS
    C, in_maps, perm, gsel = _prepare(
        x, gate_w, w1, b1, w2, shared_w1, shared_b1, shared_w2)
    nc = _build(C, STRATEGY)

    LAST_RESULTS = run_bass_kernel_spmd(nc, in_maps, core_ids=list(range(NCORES)))
    res = LAST_RESULTS.results

    b2 = np.asarray(b2)
    shared_b2 = np.asarray(shared_b2)
    s2inv = 1.0 / SCALES["s2"]
    ss2inv = 1.0 / SCALES["ss2"]
    out = np.zeros((N, D), np.float64)
    for c in range(E):
        sel = perm[c]
        yr = _unpack_yr(res[c]["yrT"], C).T[:len(sel)].astype(np.float64) * s2inv
        out[sel] += gsel[c][:, None] * (yr + b2[c].astype(np.float64))
        ys = res[c]["ysT"].astype(np.float32).reshape(NTS, D, TILE_N)
        ys2d = np.concatenate([ys[n] for n in range(NTS)], axis=1)  # [D, TS]
        out[c * TS:(c + 1) * TS] += (ys2d.T.astype(np.float64) * ss2inv
                                     + shared_b2.astype(np.float64))

    return out.reshape(B, S, D).astype(np.float32)


# revision 6
# speedup vs baseline: 2.9184x; 1.3775x over previous
"""DeepSeekMoE (top-2 of 8 experts + 2x shared expert) on 8 Trainium2 NeuronCores.

Strategy (hardcoded for x:[4,2048,2048], D=2048, H=1024, E=8, K=2):
  - Host: gating (logits/top-2/softmax) in float64 numpy; expert-parallel
    dispatch -- core e receives the tokens routed to expert e, gathered,
    transposed to [D, C] and padded to a common capacity C.  Shared-expert
    work is data-parallel: core c gets tokens [1024c, 1024(c+1)).
  - Device (SPMD): all matmuls in fp8 e4m3 with MatmulPerfMode.DoubleRow
    (K=256 contraction per instruction at 0.5 cycles/row -- ~4x fp16 MAC
    throughput).  Weights are pre-scaled by power-of-2 factors so their
    ~5e-4 magnitudes land in e4m3's normal range; the scale is divided
    out exactly in the ACT gelu (layer 1) or on the host (layer 2).
    Fully transposed dataflow (outputs [D, tokens]):
      hT = gelu(w1s.T @ xT + b1)  [fp8]; yrT = w2s.T @ hT   (routed)
      hsT = gelu(sw1s.T @ xsT + sb1);    ysT = sw2s.T @ hsT (shared)
    Layer-2 results are stored as float16 carrying the 2^s2 weight scale;
    the host divides it out in float64 during the combine.
  - Host: out[tok] += gate * (yr/s2 + b2[e]) scatter per expert; += ys/ss2 + sb2.

End-to-end absmax relative error vs float64 reference: ~1.2e-3 (fp8
operand rounding; gate is 2e-2).
"""
import contextlib
import os
import sys
import numpy as np

for _p in ("/root/.axon_site/_ro/trn_rl_repo", "/root/.axon_site/_ro/pypackages",
           "/opt/trn_rl_repo", "/opt/pypackages"):
    if os.path.isdir(_p) and _p not in sys.path:
        sys.path.append(_p)

import ml_dtypes
from concourse import bacc, mybir
from concourse import tile
from concourse.bass_utils import run_bass_kernel_spmd

NPF8 = ml_dtypes.float8_e4m3

# ---- problem constants (hardcoded per spec) ----
B, S, D, H, E = 4, 2048, 2048, 1024, 8
SH = 2 * H
N = B * S                    # 8192 tokens
TOPK = 2
NCORES = 8
TS = N // NCORES             # shared-expert tokens per core (1024)
ND = D // 128                # 16 contraction chunks over D
NH = H // 128                # 8 chunks over H
NSH = SH // 128              # 16 chunks over SH
TILE_N = 512                 # token-tile width (one PSUM bank of fp32)
NTS = TS // TILE_N           # shared-expert token tiles per core (2)

STRATEGY = "fp8"

F32 = mybir.dt.float32
F16 = mybir.dt.float16
FP8 = mybir.dt.float8e4
DRMODE = mybir.MatmulPerfMode.DoubleRowSwInterleave

LAST_RESULTS = None          # BassKernelResults of the most recent device run

_BUILD_CACHE = {}


def _swiv(wv, m):
    """SWI lhsT view of pair m of a [128, K, 128] weight tile: [128, 2, 128]
    with dim1=interleave index (stride 1), dim2=reversed column (stride 2)."""
    return (wv[:, 2 * m:2 * m + 2, :].rearrange("p a t -> p (a t)")
            .rearrange("p (t i) -> p i t", i=2))


def _ntiles(total):
    """(offset, width) token tiles covering `total` (512s, then remainder)."""
    out = []
    t = 0
    while t < total:
        w = min(TILE_N, total - t)
        out.append((t, w))
        t += w
    return out


def _build(C, strategy="fp8", loop_iters=None):
    """Build + compile the per-core SPMD program for capacity C."""
    no_store = os.environ.get("MOE_NO_STORE") == "1"     # bench-only knob
    dup_r = int(os.environ.get("MOE_DUP_R", "1"))        # bench-only knob
    dup_s = int(os.environ.get("MOE_DUP_S", "1"))        # bench-only knob
    st8 = os.environ.get("MOE_ST8") == "1"               # fp8 output stores
    STDT = FP8 if st8 else F16
    key = (C, strategy, loop_iters, no_store, dup_r, dup_s, st8)
    if key in _BUILD_CACHE:
        return _BUILD_CACHE[key]

    nc = bacc.Bacc("TRN2", target_bir_lowering=False, debug=False)

    # Weights: per-column-tile layout [ncols, 128(p), kchunks*128], element
    # (col, p, kc*128+c) = w[kc*128 + p, col*128 + c] (pre-scaled, fp8).
    # x: flat fp8, blocked per token tile: block(n) is [128(p), ND*nw] with
    # element (p, d*nw+c) = xT2d[d*128 + p, n0 + c].
    # y: flat f16, blocked per (token tile, dt): tile block [128(p), nw] =
    # yrT2d[dt*128 + p, n0:n0+nw] (carries the 2^s2 scale).
    d_xT = nc.dram_tensor("xT", [D * C], FP8, kind="ExternalInput")
    d_w1 = nc.dram_tensor("w1", [NH, 128, ND * 128], FP8, kind="ExternalInput")
    d_w2 = nc.dram_tensor("w2", [ND, 128, NH * 128], FP8, kind="ExternalInput")
    d_b1 = nc.dram_tensor("b1c", [H, 1], F32, kind="ExternalInput")
    d_xsT = nc.dram_tensor("xsT", [NTS, 128, ND * TILE_N], FP8, kind="ExternalInput")
    d_sw1 = nc.dram_tensor("sw1", [NSH, 128, ND * 128], FP8, kind="ExternalInput")
    d_sw2 = nc.dram_tensor("sw2", [ND, 128, NSH * 128], FP8, kind="ExternalInput")
    d_sb1 = nc.dram_tensor("sb1c", [SH, 1], F32, kind="ExternalInput")
    d_yrT = nc.dram_tensor("yrT", [D * C], STDT, kind="ExternalOutput")
    d_ysT = nc.dram_tensor("ysT", [NTS, ND, 128, TILE_N], STDT, kind="ExternalOutput")

    v_b1 = d_b1.ap().rearrange("(b p) o -> p b o", p=128)     # [128, NH, 1]
    v_sb1 = d_sb1.ap().rearrange("(b p) o -> p b o", p=128)   # [128, NSH, 1]

    def x_tile_view(n0, nw):
        a = d_xT.ap()[D * n0: D * (n0 + nw)]
        return a.rearrange("(p d c) -> p d c", p=128, d=ND)

    def xs_tile_view(n):
        return d_xsT.ap()[n].rearrange("p (d c) -> p d c", d=ND)

    def yr_tile_view(n0, nw, dt):
        a = d_yrT.ap()[D * n0 + dt * 128 * nw: D * n0 + (dt + 1) * 128 * nw]
        return a.rearrange("(p c) -> p c", p=128)

    def wcol_view(d_w, col, kchunks):
        return d_w.ap()[col].rearrange("p (k c) -> p k c", k=kchunks)

    gelu = mybir.ActivationFunctionType.Gelu
    rtiles = _ntiles(C)

    with tile.TileContext(nc) as tc:
        with tc.tile_pool(name="wres", bufs=1) as wres, \
             tc.tile_pool(name="xs", bufs=3) as xs, \
             tc.tile_pool(name="hp", bufs=2) as hp, \
             tc.tile_pool(name="hs", bufs=2) as hsp, \
             tc.tile_pool(name="bias", bufs=1) as bias, \
             tc.tile_pool(name="ps1", bufs=4, space="PSUM") as ps1, \
             tc.tile_pool(name="ps2", bufs=4, space="PSUM") as ps2, \
             tc.tile_pool(name="st", bufs=4) as stp:
          with (tc.For_i(0, loop_iters, 1) if loop_iters is not None
                else contextlib.nullcontext()):

            # biases first (tiny; the first gelu needs b1), then x tile 0.
            b1t = bias.tile([128, NH], F32, tag="b1")
            sb1t = bias.tile([128, NSH], F32, tag="sb1")
            nc.sync.dma_start(out=b1t[:], in_=v_b1[:, :, 0])
            nc.sync.dma_start(out=sb1t[:], in_=v_sb1[:, :, 0])

            # warm the ACT gelu table while DMAs stream: the auto-inserted
            # LoadActFuncSet binds to the first Activation in program order.
            warm = bias.tile([128, 1], F32, tag="warm")
            nc.vector.memset(warm[:], 0.0)
            nc.scalar.activation(warm[:], warm[:], gelu)

            xt0 = xs.tile([128, ND, TILE_N], FP8, tag="xstream")
            nc.sync.dma_start(out=xt0[:, :, :rtiles[0][1]],
                              in_=x_tile_view(*rtiles[0]))

            # weights resident, loaded in few big DMAs on the gpsimd queue
            # (w1 split in two so the first L1 matmuls wake up early).
            wsplit = int(os.environ.get("MOE_WSPLIT", "2"))
            def wload(d_w, ncols, kchunks, name):
                big = wres.tile([128, ncols, kchunks * 128], FP8, tag=name)
                step = max(1, ncols // wsplit)
                for c0 in range(0, ncols, step):
                    c1 = min(ncols, c0 + step)
                    nc.gpsimd.dma_start(
                        out=big[:, c0:c1, :],
                        in_=d_w.ap().rearrange("h p k -> p h k")[:, c0:c1, :])
                return [big[:, i, :].rearrange("p (k c) -> p k c", k=kchunks)
                        for i in range(ncols)]
            w1cols = wload(d_w1, NH, ND, "w1r")
            w2cols = wload(d_w2, ND, NH, "w2r")
            sw1cols = wload(d_sw1, NSH, ND, "sw1r")
            sw2cols = wload(d_sw2, ND, NSH, "sw2r")

            _store_ctr = [0]

            def store(dram_ap, psum_ap, nw):
                if no_store:
                    return
                ot = stp.tile([128, TILE_N], STDT, tag="stage")
                if _store_ctr[0] % 2 == 0:
                    nc.vector.tensor_copy(ot[:, :nw], psum_ap)
                    nc.sync.dma_start(out=dram_ap, in_=ot[:, :nw])
                else:
                    nc.scalar.activation(ot[:, :nw], psum_ap,
                                         mybir.ActivationFunctionType.Copy)
                    nc.scalar.dma_start(out=dram_ap, in_=ot[:, :nw])
                _store_ctr[0] += 1

            # ---- routed phase ----
            for n, (n0, nw) in enumerate(rtiles * dup_r):
                if n == 0:
                    xt = xt0
                else:
                    xt = xs.tile([128, ND, TILE_N], FP8, tag="xstream")
                    nc.sync.dma_start(out=xt[:, :, :nw], in_=x_tile_view(n0, nw))
                ht_t = hp.tile([128, NH, TILE_N], FP8, tag="h")
                for ht in range(NH):
                    wv = w1cols[ht]
                    pt = ps1.tile([128, TILE_N], F32, tag="p1")
                    for m in range(ND // 2):
                        nc.tensor.matmul(pt[:, :nw], lhsT=_swiv(wv, m),
                                         rhs=xt[:, 2 * m:2 * m + 2, :nw],
                                         perf_mode=DRMODE,
                                         start=(m == 0), stop=(m == ND // 2 - 1))
                    nc.scalar.activation(ht_t[:, ht, :nw], pt[:, :nw], gelu,
                                         bias=b1t[:, ht:ht + 1], scale=_S1INV[0])
                for dt in range(ND):
                    wv = w2cols[dt]
                    pt2 = ps2.tile([128, TILE_N], F32, tag="p2")
                    for m in range(NH // 2):
                        nc.tensor.matmul(pt2[:, :nw], lhsT=_swiv(wv, m),
                                         rhs=ht_t[:, 2 * m:2 * m + 2, :nw],
                                         perf_mode=DRMODE,
                                         start=(m == 0), stop=(m == NH // 2 - 1))
                    store(yr_tile_view(n0, nw, dt), pt2[:, :nw], nw)

            # ---- shared expert phase ----
            for _sdup in range(dup_s):
                for n in range(NTS):
                    xst = xs.tile([128, ND, TILE_N], FP8, tag="xstream")
                    nc.sync.dma_start(out=xst[:], in_=xs_tile_view(n))
                    hst = hsp.tile([128, NSH, TILE_N], FP8, tag="hs")
                    for st in range(NSH):
                        swv = sw1cols[st]
                        pt = ps1.tile([128, TILE_N], F32, tag="p1")
                        for m in range(ND // 2):
                            nc.tensor.matmul(pt[:], lhsT=_swiv(swv, m),
                                             rhs=xst[:, 2 * m:2 * m + 2, :],
                                             perf_mode=DRMODE,
                                             start=(m == 0), stop=(m == ND // 2 - 1))
                        nc.scalar.activation(hst[:, st, :], pt[:], gelu,
                                             bias=sb1t[:, st:st + 1], scale=_SS1INV[0])
                    for dt in range(ND):
                        swv2 = sw2cols[dt]
                        pt2 = ps2.tile([128, TILE_N], F32, tag="p2")
                        for m in range(NSH // 2):
                            nc.tensor.matmul(pt2[:], lhsT=_swiv(swv2, m),
                                             rhs=hst[:, 2 * m:2 * m + 2, :],
                                             perf_mode=DRMODE,
                                             start=(m == 0), stop=(m == NSH // 2 - 1))
                        store(d_ysT[n, dt, :, :], pt2[:], TILE_N)

    nc.compile()
    _BUILD_CACHE[key] = nc
    return nc


def _route(xf, gate_w):
    """float64 gating: top-2 indices (lax.top_k tie-break) + softmax gates."""
    logits = xf.astype(np.float64) @ np.asarray(gate_w).astype(np.float64)
    order = np.argsort(-logits, axis=1, kind="stable")
    idx = order[:, :TOPK]                                           # [N, 2]
    tl = np.take_along_axis(logits, idx, axis=1)
    tl = tl - tl.max(axis=1, keepdims=True)
    eg = np.exp(tl)
    gates = eg / eg.sum(axis=1, keepdims=True)                      # [N, 2]
    return idx, gates


def _p2scale(w):
    """Largest power of two s with absmax(w*s) <= 120 (e4m3 normal range)."""
    return 2.0 ** np.floor(np.log2(120.0 / np.abs(w).max()))


def _blockT(w, scale):
    """[K, M] weight -> scaled fp8 per-column-tile layout [M/128, 128(p), K]
    with each contraction pair m stored in DoubleRowSwInterleave order:
    free position (m, 2t+i) holds w[(2m+i)*128 + p, col*128 + (127-t)]*scale."""
    K, M = w.shape
    r = (np.asarray(w) * scale).astype(NPF8).reshape(K // 128, 128, M // 128, 128)
    b = np.ascontiguousarray(r.transpose(2, 1, 0, 3))      # [col, p, kchunk, c]
    b5 = b.reshape(M // 128, 128, K // 256, 2, 128)        # [col, p, m, i, c]
    out = np.empty((M // 128, 128, K // 256, 128, 2), NPF8)
    out[..., 0] = b5[:, :, :, 0, ::-1]
    out[..., 1] = b5[:, :, :, 1, ::-1]
    return np.ascontiguousarray(out).reshape(M // 128, 128, K)


def _pack_x(xT2d, tiles):
    """[D, C] fp8 -> flat blocked per tile: block(n) [128, ND*nw],
    (p, d*nw+c) = xT2d[d*128+p, n0+c]."""
    r = xT2d.reshape(ND, 128, xT2d.shape[1])
    parts = [np.ascontiguousarray(r[:, :, n0:n0 + nw].transpose(1, 0, 2)).ravel()
             for n0, nw in tiles]
    return np.concatenate(parts)


def _unpack_yr(flat, C):
    """Inverse of the yrT blocked layout -> [D, C] float32."""
    y = np.empty((D, C), np.float32)
    for n0, nw in _ntiles(C):
        y[:, n0:n0 + nw] = flat[D * n0: D * (n0 + nw)].reshape(D, nw)
    return y


# gelu input scales (1/s1, 1/ss1) are baked into the compiled program; they
# are set by _prepare before _build runs (module-level for _build to read).
_S1INV = [1.0]
_SS1INV = [1.0]
SCALES = {}


def _prepare(x, gate_w, w1, b1, w2, shared_w1, shared_b1, shared_w2, npdt=None):
    """Host routing + per-core input maps. Returns (C, in_maps, perm, gsel)."""
    xf = np.ascontiguousarray(np.asarray(x).reshape(N, D))
    idx, gates = _route(xf, gate_w)

    perm = []      # token ids routed to each expert (ascending)
    gsel = []      # matching gate weight
    for e in range(E):
        hit0 = idx[:, 0] == e
        hit1 = idx[:, 1] == e
        sel = np.where(hit0 | hit1)[0]
        g = np.where(hit0[sel], gates[sel, 0], gates[sel, 1])
        perm.append(sel)
        gsel.append(g)
    cmax = max(len(p) for p in perm)
    C = ((cmax + 127) // 128) * 128
    rtiles = _ntiles(C)
    stiles = _ntiles(TS)

    w1 = np.asarray(w1); w2 = np.asarray(w2)
    shared_w1 = np.asarray(shared_w1); shared_w2 = np.asarray(shared_w2)
    s1 = _p2scale(w1); s2 = _p2scale(w2)
    ss1 = _p2scale(shared_w1); ss2 = _p2scale(shared_w2)
    SCALES.update(s1=s1, s2=s2, ss1=ss1, ss2=ss2)
    _S1INV[0] = 1.0 / s1
    _SS1INV[0] = 1.0 / ss1

    xfc = xf.astype(NPF8)
    sw1b = _blockT(shared_w1, ss1)
    sw2b = _blockT(shared_w2, ss2)
    sb1c = np.ascontiguousarray(np.asarray(shared_b1).astype(np.float32)).reshape(SH, 1)
    in_maps = []
    for c in range(E):
        sel = perm[c]
        xT = np.zeros((D, C), NPF8)
        xT[:, :len(sel)] = xfc[sel].T
        xsT = np.ascontiguousarray(xfc[c * TS:(c + 1) * TS].T)
        in_maps.append({
            "xT": _pack_x(xT, rtiles),
            "w1": _blockT(w1[c], s1),
            "w2": _blockT(w2[c], s2),
            "b1c": np.ascontiguousarray(np.asarray(b1[c]).astype(np.float32)).reshape(H, 1),
            "xsT": _pack_x(xsT, stiles).reshape(NTS, 128, ND * TILE_N),
            "sw1": sw1b,
            "sw2": sw2b,
            "sb1c": sb1c,
        })
    return C, in_maps, perm, gsel


def kernel(x, gate_w, w1, b1, w2, b2, shared_w1, shared_b1, shared_w2, shared_b2):
    global LAST_RESULTS
    C, in_maps, perm, gsel = _prepare(
        x, gate_w, w1, b1, w2, shared_w1, shared_b1, shared_w2)
    nc = _build(C, STRATEGY)

    LAST_RESULTS = run_bass_kernel_spmd(nc, in_maps, core_ids=list(range(NCORES)))
    res = LAST_RESULTS.results

    b2 = np.asarray(b2)
    shared_b2 = np.asarray(shared_b2)
    s2inv = 1.0 / SCALES["s2"]
    ss2inv = 1.0 / SCALES["ss2"]
    out = np.zeros((N, D), np.float64)
    for c in range(E):
        sel = perm[c]
        yr = _unpack_yr(res[c]["yrT"], C).T[:len(sel)].astype(np.float64) * s2inv
        out[sel] += gsel[c][:, None] * (yr + b2[c].astype(np.float64))
        ys = res[c]["ysT"].astype(np.float32).reshape(NTS, D, TILE_N)
        ys2d = np.concatenate([ys[n] for n in range(NTS)], axis=1)  # [D, TS]
        out[c * TS:(c + 1) * TS] += (ys2d.T.astype(np.float64) * ss2inv
                                     + shared_b2.astype(np.float64))

    return out.reshape(B, S, D).astype(np.float32)
